# revision 1
# baseline (speedup 1.0000x reference)
"""CPC / NT-Xent loss kernel for 8 Trainium2 NeuronCores.

Reference computation (x, y: [8192, 256] f32):
    x_norm, y_norm = L2-normalized rows
    xy = concat(x_norm, y_norm)            # [16384, 256]
    sim = xy @ xy.T                        # [16384, 16384]
    denom_i = sum_j exp(sim_ij / tau) - exp(sim_ii / tau)
    pos_i   = dot(xy_i, yx_i)  (yx = concat(y_norm, x_norm))
    loss = mean( log(denom_i) - pos_i / tau )

Sharding: the 2N=16384 rows are data-parallel across the 8 cores.  Each
core receives the full row matrix ROTATED so its own 2048 rows sit at
local rows 0..2048 — the kernel is then a single SPMD program with no
core-dependent control flow.  The partner row (for pos_i) of local row i
is always local row 8192+i, independent of the rotation.

Per core, fully fused on-chip (the 16384x16384 sim matrix never touches
HBM):
  - load b [16384, 256] f32, row tiles [128, 256]
  - ss = row sums of squares (DVE scalar_tensor_tensor accum)
  - inv = rsqrt(ss) via DVE-only Newton iteration (no activation-table
    switches; the scalar engine keeps the Exp table loaded throughout)
  - rows scaled to unit norm and cast to bf16 (DVE), PE-transposed into
    PSUM collectors, copied to SBUF as bf16 B_T [256, 16384]
  - Gram row-block: for each m-tile (128 rows) x 2048-col chunk:
    bf16 matmul (K=256 = 2 accum steps) -> PSUM f32
  - ACT exp(2*psum) with accum_out giving the row-chunk sums directly;
    the (dead) exp values are written as bf16 to SBUF, which engages the
    scalar engine's 2x packed-output mode — measured faster than f32
    in-place, and keeps ACT ahead of PE so the PE never micro-idles
    (HAM stays warm)
  - denominator = rowsum - e^2  (sim_ii == 1 exactly)
  - nt_xent = ln(denominator) - 2*pos ; written out per row
The next group's load/normalize/transpose work is emitted interleaved
with the current group's matmuls so PSUM slot recycling overlaps with
compute instead of stalling at group boundaries.
Host: concatenates the 8 x 2048 per-row losses and takes the mean.
"""

import numpy as np
from contextlib import ExitStack

import concourse.bacc as bacc
import concourse.bass as bass
import concourse.tile as tile
import concourse.mybir as mybir
from concourse import bass_utils
from concourse.masks import make_identity

F32 = mybir.dt.float32
BF16 = mybir.dt.bfloat16
AF = mybir.ActivationFunctionType
ALU = mybir.AluOpType

P = 128          # partitions
TAU = 0.5
N_CORES = 8

# Full-problem geometry (hardcoded per contract)
B_ROWS = 8192    # rows in x (and y)
H = 256          # feature dim (= 2 k-tiles of 128)
N_TOTAL = 2 * B_ROWS          # 16384 rows of the concat matrix
N_MINE = N_TOTAL // N_CORES   # 2048 rows per core
CHUNK = 2048                  # columns processed per outer step (4 PSUM banks f32)

NEWTON_ITERS = 5              # rsqrt Newton steps after constant seed


class _Ctx:
    """Bag of state shared by the emission helpers."""


def build_program(n_total=N_TOTAL, n_mine=N_MINE, chunk=CHUNK, repeat=1,
                  nt_at=8, ld_at=10, exp_sbuf=True, enable_asserts=False):
    """Build the SPMD Bass program. Returns (nc, in_name, out_name).

    repeat>1 re-runs the whole computation sequentially (same math, its
    own output slice) — used to measure device time differentially when
    NTFF tracing is unavailable.
    """
    T = n_total // P              # total row tiles
    MT = n_mine // P              # my row tiles (M dimension)
    TPG = chunk // P              # row tiles ingested per outer step
    G = n_total // chunk          # outer steps
    NJ = chunk // 512             # 512-wide matmul slices per chunk
    half = T // 2                 # partner offset, in tiles
    assert H == 2 * P and half >= MT and chunk % 512 == 0
    assert n_total % chunk == 0 and n_mine % P == 0
    assert MT * P <= chunk        # lhsT slices live in the group-0 BT tile

    nc = bacc.Bacc(
        "TRN2",
        target_bir_lowering=False,
        debug=False,
        enable_asserts=enable_asserts,
        num_devices=N_CORES,
    )
    b_dram = nc.dram_tensor("b", [n_total, H], F32, kind="ExternalInput")
    # one output slice per repeat so no rep is dead code
    nt_dram = nc.dram_tensor("nt", [P, MT * repeat], F32, kind="ExternalOutput")

    with ExitStack() as ctx:
        tc = ctx.enter_context(tile.TileContext(nc))

        c = _Ctx()
        c.nc, c.b_ap, c.nt_dram = nc, b_dram.ap(), nt_dram
        c.T, c.MT, c.TPG, c.G, c.NJ, c.half, c.chunk = T, MT, TPG, G, NJ, half, chunk
        c.NT_AT, c.LD_AT = min(nt_at, MT - 2), min(ld_at, MT - 1)
        c.exp_sbuf = exp_sbuf

        c.const_pool = ctx.enter_context(tc.tile_pool(name="const", bufs=1))
        bt_pool = ctx.enter_context(tc.tile_pool(name="bt", bufs=1))
        stat_pool = ctx.enter_context(tc.tile_pool(name="stat", bufs=1))
        c.load_pool = ctx.enter_context(tc.tile_pool(name="load", bufs=3))
        c.keep_pool = ctx.enter_context(tc.tile_pool(name="keep", bufs=MT))
        c.nrm_pool = ctx.enter_context(tc.tile_pool(name="nrm", bufs=TPG + 2))
        c.sq_pool = ctx.enter_context(tc.tile_pool(name="sq", bufs=4))
        c.nwt_pool = ctx.enter_context(tc.tile_pool(name="nwt", bufs=4))
        c.exp_pool = ctx.enter_context(tc.tile_pool(name="expo", bufs=3))
        c.psum_pool = ctx.enter_context(tc.tile_pool(name="ps", bufs=2, space="PSUM"))
        c.fin_pool = ctx.enter_context(tc.tile_pool(name="fin", bufs=1))

        c.identity = c.const_pool.tile([P, P], BF16)
        make_identity(nc, c.identity[:])

        # B_T: normalized rows, transposed, bf16.  Two k-halves, one tile
        # per column group (separate tiles -> no false cross-group deps).
        c.BT0s = [bt_pool.tile([P, chunk], BF16, tag=f"bt0_{g}", name=f"bt0_{g}")
                  for g in range(G)]
        c.BT1s = [bt_pool.tile([P, chunk], BF16, tag=f"bt1_{g}", name=f"bt1_{g}")
                  for g in range(G)]

        c.ss_all = stat_pool.tile([P, T], F32)     # row sums of squares
        c.inv_all = stat_pool.tile([P, T], F32)    # 1/norm
        c.rs_all = stat_pool.tile([P, MT * G], F32)  # exp sums, col = m*G+g
        c.pos_all = stat_pool.tile([P, MT], F32)   # pos_sim per my-row

        for rep in range(repeat):
            emit_rep(c, rep)

    nc.compile()
    return nc, "b", "nt"


def emit_loads(c, g):
    """DMA the whole group as ONE slab + row sums of squares + Newton
    rsqrt.  A single 2MB DMA instead of 16 x 128KB cuts per-DMA queue
    overhead and sequencer/semaphore traffic."""
    nc = c.nc
    slab = c.load_pool.tile([P, c.TPG, H], F32, tag="raw", name=f"slab_{g}")
    src = c.b_ap[g * c.TPG * P:(g + 1) * c.TPG * P, :].rearrange(
        "(t p) m -> p t m", p=P)
    nc.sync.dma_start(out=slab[:], in_=src)
    raws = []
    for t in range(c.TPG):
        ti = g * c.TPG + t
        raw = slab[:, t, :]
        sq = c.sq_pool.tile([P, H], F32, tag="sq", name="sqs")
        nc.vector.scalar_tensor_tensor(
            out=sq[:], in0=raw, scalar=1.0, in1=raw,
            op0=ALU.mult, op1=ALU.mult,
            accum_out=c.ss_all[:, ti:ti + 1],
        )
        raws.append(raw)
    c.raws[g] = raws

    # inv = rsqrt(ss), DVE-only Newton iteration.
    # seed y0 = H**-0.5 (ss concentrates near H for unit-variance rows);
    # y1 = y0*(1.5 - 0.5*y0^2*ss) folds into one tensor_scalar op.
    u = c.ss_all[:, g * c.TPG:(g + 1) * c.TPG]
    y0 = float(H) ** -0.5
    y = c.nwt_pool.tile([P, c.TPG], F32, tag="nwty", name="nwty")
    nc.vector.tensor_scalar(
        out=y[:], in0=u, scalar1=-0.5 * y0 ** 3, scalar2=1.5 * y0,
        op0=ALU.mult, op1=ALU.add)
    inv_slice = c.inv_all[:, g * c.TPG:(g + 1) * c.TPG]
    for it in range(NEWTON_ITERS - 1):
        t1 = c.nwt_pool.tile([P, c.TPG], F32, tag="nwtt", name="nwtt")
        nc.vector.scalar_tensor_tensor(
            out=t1[:], in0=y[:], scalar=1.0, in1=y[:],
            op0=ALU.mult, op1=ALU.mult)               # y^2
        t2 = c.nwt_pool.tile([P, c.TPG], F32, tag="nwtt2", name="nwtt2")
        nc.vector.scalar_tensor_tensor(
            out=t2[:], in0=u, scalar=-0.5, in1=t1[:],
            op0=ALU.mult, op1=ALU.mult)               # -0.5*ss*y^2
        last = it == NEWTON_ITERS - 2
        ynew = inv_slice if last else c.nwt_pool.tile(
            [P, c.TPG], F32, tag="nwty", name="nwty")
        nc.vector.scalar_tensor_tensor(
            out=ynew if last else ynew[:], in0=t2[:], scalar=1.5, in1=y[:],
            op0=ALU.add, op1=ALU.mult)                # y*(1.5 - 0.5 ss y^2)
        y = ynew if not last else None


def emit_normalize_transpose(c, g):
    """Scale group-g rows to unit norm (bf16), compute pos dots,
    PE-transpose into PSUM collectors and copy to the bf16 B_T tiles."""
    nc = c.nc
    psA = c.psum_pool.tile([P, c.chunk], BF16, tag="ps", name="psA")
    psB = c.psum_pool.tile([P, c.chunk], BF16, tag="ps", name="psB")
    for t in range(c.TPG):
        ti = g * c.TPG + t
        if ti < c.MT:
            nrm = c.keep_pool.tile([P, H], BF16, tag="keep", name=f"keep_{ti}")
        else:
            nrm = c.nrm_pool.tile([P, H], BF16, tag="nrm", name="nrm")
        nc.vector.tensor_scalar_mul(nrm[:], c.raws[g][t][:],
                                    c.inv_all[:, ti:ti + 1])
        if ti < c.MT:
            c.kept[ti] = nrm
        if c.half <= ti < c.half + c.MT:
            m = ti - c.half
            sq2 = c.sq_pool.tile([P, H], F32, tag="sq2", name="sq2")
            nc.vector.scalar_tensor_tensor(
                out=sq2[:], in0=nrm[:], scalar=1.0, in1=c.kept[m][:],
                op0=ALU.mult, op1=ALU.mult,
                accum_out=c.pos_all[:, m:m + 1],
            )
        nc.tensor.transpose(psA[:, t * P:(t + 1) * P], nrm[:, 0:P],
                            c.identity[:])
        nc.tensor.transpose(psB[:, t * P:(t + 1) * P], nrm[:, P:2 * P],
                            c.identity[:])
    del c.raws[g]
    # copy PSUM collectors into B_T (bf16, DVE 2x mode)
    nc.vector.tensor_copy(out=c.BT0s[g][:], in_=psA[:])
    nc.vector.tensor_copy(out=c.BT1s[g][:], in_=psB[:])


def emit_rep(c, rep):
    nc = c.nc
    c.kept = [None] * c.MT
    c.raws = {}

    emit_loads(c, 0)
    emit_normalize_transpose(c, 0)
    if c.G > 1:
        emit_loads(c, 1)

    for g in range(c.G):
        for m in range(c.MT):
            ps = c.psum_pool.tile([P, c.chunk], F32, tag="ps", name="mm_ps")
            lhs0 = c.BT0s[0][:, m * P:(m + 1) * P]
            lhs1 = c.BT1s[0][:, m * P:(m + 1) * P]
            for j in range(c.NJ):
                nc.tensor.matmul(
                    ps[:, j * 512:(j + 1) * 512], lhs0,
                    c.BT0s[g][:, j * 512:(j + 1) * 512],
                    start=True, stop=False)
            for j in range(c.NJ):
                nc.tensor.matmul(
                    ps[:, j * 512:(j + 1) * 512], lhs1,
                    c.BT1s[g][:, j * 512:(j + 1) * 512],
                    start=False, stop=True)
            # exp values are dead; only accum_out matters.  bf16 SBUF out
            # hits the ACT 2x output mode (vs 1x for f32-in-place), keeping
            # ACT faster than PE so the PE never micro-idles (HAM warm).
            acc = c.rs_all[:, m * c.G + g: m * c.G + g + 1]
            if c.exp_sbuf:
                eo = c.exp_pool.tile([P, c.chunk], BF16, tag="eo", name="eo")
                nc.scalar.activation(
                    out=eo[:], in_=ps[:], func=AF.Exp, scale=2.0, accum_out=acc)
            else:
                nc.scalar.activation(
                    out=ps[:], in_=ps[:], func=AF.Exp, scale=2.0, accum_out=acc)
            # interleave next group's prologue into this group's mm stream
            if m == c.NT_AT and g + 1 < c.G:
                emit_normalize_transpose(c, g + 1)
            if m == c.LD_AT and g + 2 < c.G:
                emit_loads(c, g + 2)

    # ---- finalize ----
    MT, G = c.MT, c.G
    rowsum = c.fin_pool.tile([P, MT], F32, tag="rowsum", name="rowsum")
    nc.vector.tensor_reduce(
        out=rowsum[:], in_=c.rs_all[:].rearrange("p (m g) -> p m g", g=G),
        axis=mybir.AxisListType.X, op=ALU.add)
    denom = c.fin_pool.tile([P, MT], F32, tag="denom", name="denom")
    nc.vector.tensor_scalar_add(denom[:], rowsum[:], -float(np.exp(2.0)))
    lnd = c.fin_pool.tile([P, MT], F32, tag="lnd", name="lnd")
    nc.scalar.activation(out=lnd[:], in_=denom[:], func=AF.Ln)
    ntv = c.fin_pool.tile([P, MT], F32, tag="ntv", name="ntv")
    # nt = (pos * -2) + ln(denom)
    nc.vector.scalar_tensor_tensor(
        out=ntv[:], in0=c.pos_all[:], scalar=-2.0, in1=lnd[:],
        op0=ALU.mult, op1=ALU.add)
    nc.sync.dma_start(
        out=c.nt_dram.ap()[:, rep * MT:(rep + 1) * MT], in_=ntv[:])


_CACHE = {}


def _get_program():
    if "nc" not in _CACHE:
        _CACHE["nc"] = build_program()
    return _CACHE["nc"]


def kernel(x: np.ndarray, y: np.ndarray) -> np.ndarray:
    x = np.asarray(x, dtype=np.float32)
    y = np.asarray(y, dtype=np.float32)
    xy = np.concatenate([x, y], axis=0)          # [16384, 256]

    nc, in_name, out_name = _get_program()

    in_maps = []
    for c in range(N_CORES):
        off = c * N_MINE
        b_rot = np.ascontiguousarray(np.roll(xy, -off, axis=0))
        in_maps.append({in_name: b_rot})

    res = bass_utils.run_bass_kernel_spmd(
        nc, in_maps, core_ids=list(range(N_CORES)))

    # nt[c][p, m] = loss for global row (c*N_MINE + m*128 + p)
    rows = np.concatenate(
        [res.results[c][out_name].T.reshape(-1) for c in range(N_CORES)])
    loss = rows.astype(np.float64).mean()
    return np.float32(loss)



# revision 4
# speedup vs baseline: 4.3397x; 4.3397x over previous
"""CPC loss kernel v3: symmetry-halved Gram computation.

sim = B B^T is symmetric: each unordered block-pair {a,b} is computed once.
Core-local rows are processed in PAIRS (2r, 2r+1); both rows of a pair share
the column strip [2r, 2r+65] (66 tiles).  A computed exp-block contributes
its row sums (ACT accum_out) to the block-row AND its column sums (fp8
DoubleRow ones-matmul over the pair's two exp planes) to the mirrored rows.
Strip-edge tiles (strip index {0, 1, 64, 65}) are double-covered globally,
so their exp is scaled by 0.5 via the activation bias (exp(2s - ln2)).

Per core the work is the 16 x 80-tile band instead of 16 x 128 tiles:
  - PE: DoubleRow fp8 sim matmuls (K=256 in one pass) + paired colsum
    ones-matmuls + transposes  (~35% less than full row-panel)
  - ACT: fp8-output exp (4x mode) with accum row sums; B_T psum->fp8 copies
  - DVE: squares, Newton rsqrt, normalize, pos dots, eo edge memsets
Outputs per core: rs [128,16] (row-half partial denominators), pos [128,16],
cs [NJ,1024] (column-half partials, band-indexed).  The host assembles
denominators across cores, does ln, and means (16K-element numpy work).

Inputs per core: only the 80-tile band (10240 rows) of the rotated concat
matrix -> 10.5MB HBM traffic instead of 16.8MB.
"""

import math
import numpy as np
from contextlib import ExitStack

import concourse.bacc as bacc
import concourse.bass as bass
import concourse.tile as tile
import concourse.mybir as mybir
from concourse import bass_utils
from concourse.masks import make_identity

F32 = mybir.dt.float32
BF16 = mybir.dt.bfloat16
FP8 = mybir.dt.float8e4
AF = mybir.ActivationFunctionType
ALU = mybir.AluOpType
DR = mybir.MatmulPerfMode.DoubleRow

P = 128
TAU = 0.5
N_CORES = 8
LN2 = float(np.log(2.0))

B_ROWS = 8192
H = 256
N_TOTAL = 2 * B_ROWS
N_MINE = N_TOTAL // N_CORES

NEWTON_ITERS = 5
JT = 8                      # band tiles per column chunk


class _Ctx:
    pass


def geometry(n_total, n_mine):
    T = n_total // P
    MT = n_mine // P
    R = MT // 2
    HalfT = T // 2
    SL = HalfT + 2            # strip length in tiles
    NB = MT + HalfT           # band tiles
    NJ = (NB + JT - 1) // JT
    return T, MT, R, HalfT, SL, NB, NJ


def build_program(n_total=N_TOTAL, n_mine=N_MINE, repeat=1,
                  enable_asserts=False, loop_trips=1):
    T, MT, R, HalfT, SL, NB, NJ = geometry(n_total, n_mine)
    assert H == 2 * P and MT % 2 == 0 and JT <= HalfT
    assert MT <= 2 * JT, "lhsT tiles must sit in the first two BT chunks"

    nc = bacc.Bacc(
        "TRN2",
        target_bir_lowering=False,
        debug=False,
        enable_asserts=enable_asserts,
        num_devices=N_CORES,
    )
    b_dram = nc.dram_tensor("b", [NB * P, H], F32, kind="ExternalInput")
    rs_dram = nc.dram_tensor("rs", [P, MT * repeat], F32, kind="ExternalOutput")
    pos_dram = nc.dram_tensor("pos", [P, MT * repeat], F32, kind="ExternalOutput")
    cs_dram = nc.dram_tensor("cs", [NJ * repeat, JT * P], F32,
                             kind="ExternalOutput")

    with ExitStack() as ctx:
        tc = ctx.enter_context(tile.TileContext(nc))

        c = _Ctx()
        c.nc, c.b_ap = nc, b_dram.ap()
        c.rs_dram, c.pos_dram, c.cs_dram = rs_dram, pos_dram, cs_dram
        c.T, c.MT, c.R, c.HalfT, c.SL, c.NB, c.NJ = T, MT, R, HalfT, SL, NB, NJ

        c.const_pool = ctx.enter_context(tc.tile_pool(name="const", bufs=1))
        bt_pool = ctx.enter_context(tc.tile_pool(name="btp", bufs=1))
        c.btr_pool = ctx.enter_context(tc.tile_pool(name="btr", bufs=3))
        stat_pool = ctx.enter_context(tc.tile_pool(name="stat", bufs=1))
        c.load_pool = ctx.enter_context(tc.tile_pool(name="load", bufs=3))
        c.keep_pool = ctx.enter_context(tc.tile_pool(name="keep", bufs=MT))
        c.nrm_pool = ctx.enter_context(tc.tile_pool(name="nrm", bufs=6))
        c.sq_pool = ctx.enter_context(tc.tile_pool(name="sq", bufs=4))
        c.nwt_pool = ctx.enter_context(tc.tile_pool(name="nwt", bufs=4))
        c.eo_pool = ctx.enter_context(tc.tile_pool(name="eo", bufs=3))
        c.psum_pool = ctx.enter_context(tc.tile_pool(name="ps", bufs=2,
                                                     space="PSUM"))
        c.cs_pool = ctx.enter_context(tc.tile_pool(name="cs", bufs=2,
                                                   space="PSUM"))
        c.fin_pool = ctx.enter_context(tc.tile_pool(name="fin", bufs=1))

        c.identity = c.const_pool.tile([P, P], BF16)
        make_identity(nc, c.identity[:])
        # [128, 2, 16] so the plane stride is 16B (dual-fp8 ldweights
        # requires even, 16B-aligned outer steps); lhsT slices [:, :, 0:1].
        c.ones8 = c.const_pool.tile([P, 2, 16], FP8)
        nc.vector.memset(c.ones8[:], 1.0)
        c.negln2 = c.const_pool.tile([P, 1], F32)
        nc.vector.memset(c.negln2[:], -LN2)

        # persistent BT chunks (hold the lhsT row tiles 0..MT-1)
        n_persist = (MT + JT - 1) // JT
        c.BTp = [bt_pool.tile([P, 2, JT * P], FP8, tag=f"btp{j}",
                              name=f"btp{j}") for j in range(n_persist)]
        c.n_persist = n_persist

        c.ss_all = stat_pool.tile([P, NB], F32)
        c.inv_all = stat_pool.tile([P, NB], F32)
        c.rs_all = stat_pool.tile([P, MT * NJ * 2], F32)
        c.pos_all = stat_pool.tile([P, MT], F32)
        c.junk = stat_pool.tile([P, 8], F32)

        if loop_trips > 1:
            with tc.For_i(0, loop_trips) as _i:
                emit_rep(c, 0)
        else:
            for rep in range(repeat):
                emit_rep(c, rep)

    nc.compile()
    return nc, "b", ("rs", "pos", "cs")


def g_tiles(c, j):
    j0 = j * JT
    j1 = min(c.NB, j0 + JT)
    return j0, j1


def emit_load_dma(c, j):
    nc = c.nc
    j0, j1 = g_tiles(c, j)
    tj = j1 - j0
    slab = c.load_pool.tile([P, tj, H], F32, tag="raw", name=f"slab_{j}")
    src = c.b_ap[j0 * P:j1 * P, :].rearrange("(t p) m -> p t m", p=P)
    nc.sync.dma_start(out=slab[:], in_=src)
    c.slabs[j] = slab


def emit_squares(c, j):
    # row sums of squares + Newton rsqrt for group j; the slab was DMA'd a
    # full chunk earlier so the DVE queue never head-of-line blocks on it
    nc = c.nc
    j0, j1 = g_tiles(c, j)
    tj = j1 - j0
    slab = c.slabs.pop(j)
    raws = []
    for t in range(tj):
        ti = j0 + t
        raw = slab[:, t, :]
        sq = c.sq_pool.tile([P, H], F32, tag="sq", name="sqs")
        nc.vector.scalar_tensor_tensor(
            out=sq[:], in0=raw, scalar=1.0, in1=raw,
            op0=ALU.mult, op1=ALU.mult,
            accum_out=c.ss_all[:, ti:ti + 1],
        )
        raws.append(raw)
    c.raws[j] = raws

    u = c.ss_all[:, j0:j1]
    y0 = float(H) ** -0.5
    y = c.nwt_pool.tile([P, tj], F32, tag="nwty", name="nwty")
    nc.vector.tensor_scalar(
        out=y[:], in0=u, scalar1=-0.5 * y0 ** 3, scalar2=1.5 * y0,
        op0=ALU.mult, op1=ALU.add)
    inv_slice = c.inv_all[:, j0:j1]
    for it in range(NEWTON_ITERS - 1):
        t1 = c.nwt_pool.tile([P, tj], F32, tag="nwtt", name="nwtt")
        nc.vector.scalar_tensor_tensor(
            out=t1[:], in0=y[:], scalar=1.0, in1=y[:],
            op0=ALU.mult, op1=ALU.mult)
        t2 = c.nwt_pool.tile([P, tj], F32, tag="nwtt2", name="nwtt2")
        nc.vector.scalar_tensor_tensor(
            out=t2[:], in0=u, scalar=-0.5, in1=t1[:],
            op0=ALU.mult, op1=ALU.mult)
        last = it == NEWTON_ITERS - 2
        ynew = inv_slice if last else c.nwt_pool.tile(
            [P, tj], F32, tag="nwty", name="nwty")
        nc.vector.scalar_tensor_tensor(
            out=ynew if last else ynew[:], in0=t2[:], scalar=1.5, in1=y[:],
            op0=ALU.add, op1=ALU.mult)
        y = ynew if not last else None


def emit_normalize(c, j):
    # normalize group-j tiles (DVE, bf16) + pos dots; transposes are emitted
    # separately (later) so the PE queue never waits on this DVE work
    nc = c.nc
    j0, j1 = g_tiles(c, j)
    tj = j1 - j0
    if j < c.n_persist:
        BT = c.BTp[j]
    else:
        BT = c.btr_pool.tile([P, 2, JT * P], FP8, tag="btr", name=f"bt_{j}")
    c.BTs[j] = BT
    nrms = []
    for t in range(tj):
        ti = j0 + t
        if ti < c.MT:
            nrm = c.keep_pool.tile([P, H], BF16, tag="keep", name=f"keep_{ti}")
            c.kept[ti] = nrm
        else:
            nrm = c.nrm_pool.tile([P, H], BF16, tag="nrm", name="nrm")
        nc.vector.tensor_scalar_mul(nrm[:], c.raws[j][t][:],
                                    c.inv_all[:, ti:ti + 1])
        if c.HalfT <= ti < c.HalfT + c.MT:
            m = ti - c.HalfT
            sq2 = c.sq_pool.tile([P, H], BF16, tag="sq2", name="sq2")
            nc.vector.scalar_tensor_tensor(
                out=sq2[:], in0=nrm[:], scalar=1.0, in1=c.kept[m][:],
                op0=ALU.mult, op1=ALU.mult,
                accum_out=c.pos_all[:, m:m + 1],
            )
        nrms.append(nrm)
    del c.raws[j]
    c.nrms[j] = nrms


def emit_transposes(c, j):
    # PE-transpose group-j rows in rounds of 4 tiles -> [P, 512] collectors
    # -> ACT-copy into the fp8 BT chunk planes
    nc = c.nc
    j0, j1 = g_tiles(c, j)
    tj = j1 - j0
    BT = c.BTs[j]
    nrms = c.nrms.pop(j)
    for r0 in range(0, tj, 4):
        r1 = min(tj, r0 + 4)
        w = (r1 - r0) * P
        psA = c.psum_pool.tile([P, 512], BF16, tag="col", name="psA")
        psB = c.psum_pool.tile([P, 512], BF16, tag="col", name="psB")
        for t in range(r0, r1):
            nc.tensor.transpose(psA[:, (t - r0) * P:(t - r0 + 1) * P],
                                nrms[t][:, 0:P], c.identity[:])
            nc.tensor.transpose(psB[:, (t - r0) * P:(t - r0 + 1) * P],
                                nrms[t][:, P:2 * P], c.identity[:])
        nc.scalar.activation(out=BT[:, 0, r0 * P:r0 * P + w],
                             in_=psA[:, 0:w], func=AF.Copy,
                             accum_out=c.junk[:, 0:1])
        nc.scalar.activation(out=BT[:, 1, r0 * P:r0 * P + w],
                             in_=psB[:, 0:w], func=AF.Copy,
                             accum_out=c.junk[:, 1:2])


def _split512(c0, c1):
    """Split [c0, c1) at 512-aligned boundaries."""
    segs = []
    while c0 < c1:
        nxt = min(c1, (c0 // 512 + 1) * 512)
        segs.append((c0, nxt))
        c0 = nxt
    return segs


def emit_J(c, J, rep):
    nc = c.nc
    j0, j1 = g_tiles(c, J)
    W = (j1 - j0) * P
    pairs = [r for r in range(c.R)
             if 2 * r <= j1 - 1 and 2 * r + c.SL - 1 >= j0]
    eo = c.eo_pool.tile([P, c.MT, JT * P], FP8, tag="eo", name=f"eo_{J}")

    # zero uncovered plane edges of partially-covered pairs
    for r in pairs:
        a = max(2 * r, j0)
        b = min(2 * r + c.SL - 1, j1 - 1)
        if a > j0:
            nc.vector.memset(eo[:, 2 * r:2 * r + 2, 0:(a - j0) * P], 0.0)
        if b < j1 - 1:
            nc.vector.memset(
                eo[:, 2 * r:2 * r + 2, (b + 1 - j0) * P:W], 0.0)

    for ri, r in enumerate(pairs):
        a = max(2 * r, j0)
        b = min(2 * r + c.SL - 1, j1 - 1)
        c0 = (a - j0) * P
        c1 = (b + 1 - j0) * P
        for p in (0, 1):
            m = 2 * r + p
            lhs_bt = c.BTp[m // JT]
            lhs = lhs_bt[:, :, (m % JT) * P:(m % JT + 1) * P]
            mm = c.psum_pool.tile([P, JT * P], F32, tag="mm", name="mm_ps")
            for s0, s1 in _split512(c0, c1):
                nc.tensor.matmul(mm[:, s0:s1], lhs,
                                 c.BTs[J][:, :, s0:s1],
                                 start=True, stop=True, perf_mode=DR)
            # biased (0.5x) strip-edge tiles: band tiles {2r, 2r+1} and
            # {2r+HalfT, 2r+HalfT+1}, each fully inside one J chunk
            lb0, lb1 = 2 * r, 2 * r + 1                  # left-biased tiles
            rb0, rb1 = 2 * r + c.HalfT, 2 * r + c.HalfT + 1
            segs = []   # (colstart, colend, biased)
            if j0 <= lb0 and lb1 <= j1 - 1:
                assert a == lb0
                segs.append((c0, c0 + 2 * P, True))
                segs.append((c0 + 2 * P, c1, False))
            elif j0 <= rb0 and rb1 <= j1 - 1:
                assert b == rb1
                segs.append((c0, c1 - 2 * P, False))
                segs.append((c1 - 2 * P, c1, True))
            else:
                segs.append((c0, c1, False))
            for si, (s0, s1, biased) in enumerate(segs):
                if s0 >= s1:
                    continue
                slot = (m * c.NJ + J) * 2 + si
                nc.scalar.activation(
                    out=eo[:, m, s0:s1], in_=mm[:, s0:s1], func=AF.Exp,
                    scale=2.0, bias=(c.negln2[:] if biased else 0.0),
                    accum_out=c.rs_all[:, slot:slot + 1])
        # previous chunk's colsum: by now its last exp has long finished,
        # so the PE never stalls waiting on ACT
        if ri == 0 and c.pending_colsum is not None:
            c.pending_colsum()
            c.pending_colsum = None
        # software-pipeline the next groups' prologue between pairs
        if ri == min(1, len(pairs) - 1) and J + 1 < c.NJ:
            emit_squares(c, J + 1)
            emit_normalize(c, J + 1)
        if ri == min(3, len(pairs) - 1) and J + 2 < c.NJ:
            emit_load_dma(c, J + 2)
        if ri == min(5, len(pairs) - 1) and J + 1 < c.NJ:
            emit_transposes(c, J + 1)

    # column sums (deferred into the next J's pair loop): per 512-segment,
    # chain fp8 DoubleRow ones-matmuls over all pairs into one [1, 512]
    # PSUM accumulator; stage via SBUF (PSUM cannot DMA directly) and DMA
    # the J row out once.
    def colsum():
        cs_sb = c.fin_pool.tile([1, JT * P], F32, tag="cs_sb", name="cs_sb")
        for s0, s1 in _split512(0, W):
            cs = c.cs_pool.tile([1, 512], F32, tag="cs", name="cs")
            for ri, r in enumerate(pairs):
                nc.tensor.matmul(
                    cs[0:1, 0:s1 - s0], c.ones8[:, :, 0:1],
                    eo[:, 2 * r:2 * r + 2, s0:s1],
                    start=(ri == 0), stop=(ri == len(pairs) - 1),
                    perf_mode=DR)
            nc.vector.tensor_copy(out=cs_sb[0:1, s0:s1],
                                  in_=cs[0:1, 0:s1 - s0])
        nc.sync.dma_start(
            out=c.cs_dram.ap()[rep * c.NJ + J:rep * c.NJ + J + 1, 0:W],
            in_=cs_sb[0:1, 0:W])
    if c.pending_colsum is not None:
        c.pending_colsum()
    c.pending_colsum = colsum


def emit_rep(c, rep):
    nc = c.nc
    c.kept = [None] * c.MT
    c.raws = {}
    c.BTs = {}
    c.nrms = {}
    c.slabs = {}
    c.pending_colsum = None
    nc.vector.memset(c.rs_all[:], 0.0)

    emit_load_dma(c, 0)
    if c.NJ > 1:
        emit_load_dma(c, 1)
    emit_squares(c, 0)
    emit_normalize(c, 0)
    emit_transposes(c, 0)

    for J in range(c.NJ):
        emit_J(c, J, rep)
    if c.pending_colsum is not None:
        c.pending_colsum()
        c.pending_colsum = None

    MT, NJ = c.MT, c.NJ
    rowsum = c.fin_pool.tile([P, MT], F32, tag="rowsum", name="rowsum")
    nc.vector.tensor_reduce(
        out=rowsum[:],
        in_=c.rs_all[:].rearrange("p (m j) -> p m j", j=NJ * 2),
        axis=mybir.AxisListType.X, op=ALU.add)
    nc.sync.dma_start(
        out=c.rs_dram.ap()[:, rep * MT:(rep + 1) * MT], in_=rowsum[:])
    nc.sync.dma_start(
        out=c.pos_dram.ap()[:, rep * MT:(rep + 1) * MT], in_=c.pos_all[:])


_CACHE = {}


def _get_program():
    if "nc" not in _CACHE:
        _CACHE["nc"] = build_program()
    return _CACHE["nc"]


def combine(rs, pos, cs, n_total=N_TOTAL, n_mine=N_MINE):
    """Host-side assembly: rs/pos/cs are [n_cores, ...] stacked outputs."""
    T, MT, R, HalfT, SL, NB, NJ = geometry(n_total, n_mine)
    n_cores = rs.shape[0]
    denom = np.zeros(n_total, dtype=np.float64)
    posv = np.zeros(n_total, dtype=np.float64)
    for core in range(n_cores):
        base = core * MT
        # row-half partials + pos: local row (m, p) -> global row
        gtile = (base + np.arange(MT)) % T
        gidx = (gtile[:, None] * P + np.arange(P)[None, :]).reshape(-1)
        denom[gidx] += rs[core].T.astype(np.float64).reshape(-1)
        posv[gidx] = pos[core].T.astype(np.float64).reshape(-1)
        # column-half partials: band col idx -> global row
        cs_flat = cs[core].astype(np.float64).reshape(-1)[:NB * P]
        btile = (base + np.arange(NB)) % T
        bidx = (btile[:, None] * P + np.arange(P)[None, :]).reshape(-1)
        np.add.at(denom, bidx, cs_flat)
    denom = denom - np.exp(2.0)
    nt = np.log(denom) - 2.0 * posv
    return nt.mean()


def kernel(x: np.ndarray, y: np.ndarray) -> np.ndarray:
    x = np.asarray(x, dtype=np.float32)
    y = np.asarray(y, dtype=np.float32)
    xy = np.concatenate([x, y], axis=0)

    nc, in_name, out_names = _get_program()
    T, MT, R, HalfT, SL, NB, NJ = geometry(N_TOTAL, N_MINE)

    in_maps = []
    for c in range(N_CORES):
        off = c * N_MINE
        band = np.take(xy, (off + np.arange(NB * P)) % N_TOTAL, axis=0)
        in_maps.append({in_name: np.ascontiguousarray(band)})

    res = bass_utils.run_bass_kernel_spmd(
        nc, in_maps, core_ids=list(range(N_CORES)))

    rs = np.stack([res.results[c]["rs"] for c in range(N_CORES)])
    pos = np.stack([res.results[c]["pos"] for c in range(N_CORES)])
    cs = np.stack([res.results[c]["cs"] for c in range(N_CORES)])
    return np.float32(combine(rs, pos, cs))
